# revision 1
# baseline (speedup 1.0000x reference)
"""CodonAttention Trainium2 kernel.

Math (per batch b, head h):
  q = x @ wq.T + bq ; k = x @ wk.T + bk ; v = x @ wv.T + bv   (head slices)
  scores = q k^T / 8 + syn_bias[codons_i, codons_j]
  out    = softmax(scores) @ v ;  final = concat_heads(out) @ wo.T + bo

Key algebraic trick: the pairwise codon bias factors through one-hots,
  pair_bias = onehot @ syn_bias @ onehot.T
so augmenting q' = [(q+bq)/8 | onehot @ syn_bias] and k' = [k | onehot] gives
  scores = q' @ k'.T        (effective head dim 128 — exactly one partition)
Softmax runs without max-subtraction (|scores| <= ~4.3, exp is safe in fp32),
and the row-sum l is obtained by appending a ones-column to v:
  [O | l] = P @ [v | 1].  The ones column comes free: wvT's padded column is
zero, and the per-partition bias column [bv | 1 | 0..] sets it during the
PSUM->SBUF eviction.

Sharding: 8 cores = (batch b in {0,1}) x (head h in {0..3}). Each core runs
the full attention for its (b, h) and produces the UNNORMALIZED partial
output projection outT = (wo_h @ O_h.T) in (256, 4096) layout plus the
softmax denominators lT (1, 4096); the host divides, sums the 4 head
partials per batch, transposes, and adds bo. Host-side division keeps the
single-partition reciprocal (3.4us/block on DVE) off the device's critical
path.

Layout/engine choices driven by the profile:
- All big matmuls float32r (fp32 with 11-bit-rounded mantissa): 1 cycle/row
  on the PE at moving-dim >= 256 vs 4 cycles/row for full fp32. Producers
  feeding fp32r matmuls must output fp32r; DRAM operands are pre-rounded on
  the host (round-half-up at mantissa bit 12, bit-identical to walrus).
- v is computed transposed (vT, N=512 moving dim) then flipped to key-major
  with TensorE transposes — computing v directly needs N=68 matmuls which
  run at 4 cycles/row.
- The attention stream is software-pipelined: score matmuls of group g+1
  are emitted before the PV matmuls of group g so the PE computes scores
  while ACT exponentiates; the per-block output projection is emitted
  inside the stream so output DMA overlaps compute.
"""

import numpy as np

import concourse.mybir as mybir
import concourse.tile as tile
from concourse import bacc
from concourse.bass_utils import run_bass_kernel_spmd


def _ensure_axon_ntff_hook():
    """This image's antenv package lacks axon_hooks, so
    run_bass_kernel_spmd(trace=True) (or BASS_TRACE=1) would die on the
    import. Register a compatible module backed by the libaxon_pjrt C ABI
    so tracing works if a caller requests it."""
    import sys
    try:
        import antenv.axon_hooks  # noqa: F401
        return
    except ImportError:
        pass
    import contextlib
    import ctypes
    import types
    try:
        lib = ctypes.CDLL("/opt/axon/libaxon_pjrt.so")
        has = hasattr(lib, "axon_start_nrt_profile")
    except OSError:
        has = False
    if has:
        lib.axon_start_nrt_profile.argtypes = [ctypes.POINTER(ctypes.c_int64),
                                               ctypes.c_size_t]
        lib.axon_start_nrt_profile.restype = ctypes.c_int64
        lib.axon_stop_nrt_profile.argtypes = [ctypes.c_char_p]
        lib.axon_stop_nrt_profile.restype = ctypes.c_int64

        @contextlib.contextmanager
        def _hook(output_dir, device_ids):
            import jax
            jax.devices()
            if device_ids:
                ids = (ctypes.c_int64 * len(device_ids))(*device_ids)
                rc = lib.axon_start_nrt_profile(ids, len(device_ids))
            else:
                rc = lib.axon_start_nrt_profile(None, 0)
            if rc != 0:
                raise RuntimeError(f"axon_start_nrt_profile rc={rc}")
            try:
                yield
            finally:
                lib.axon_stop_nrt_profile(str(output_dir).encode())
    else:
        _hook = None

    mod = types.ModuleType("antenv.axon_hooks")
    _state = {"hook": _hook}
    mod.get_axon_ntff_profile_hook = lambda: _state["hook"]
    mod.set_axon_ntff_profile_hook = lambda h: _state.__setitem__("hook", h)
    sys.modules["antenv.axon_hooks"] = mod


_ensure_axon_ntff_hook()

B, S, HID, NH, D = 2, 4096, 256, 4, 64
DV = D + 4         # v + ones column + 3 cols fp32r-alignment padding
LCOL = D           # index of the ones column inside a v tile
QB = 512           # query block (free dim of score matmuls)
KT = 128           # key tile (partition dim of transposed scores)
NQB = S // QB      # 8
NKT = S // KT      # 32
GRP = 2            # key tiles per exp group (2 PSUM banks per group)

F32 = mybir.dt.float32
F32R = mybir.dt.float32r
Exp = mybir.ActivationFunctionType.Exp


def round_fp32r(a):
    """Round-half-up at mantissa bit 12 — bit-identical to walrus
    fp32_to_fp32r (verified against libwalrus on 20k samples)."""
    a = np.ascontiguousarray(a, np.float32)
    u = a.view(np.uint32).astype(np.uint64)
    return (((u + 0x800) & 0xFFFFF000).astype(np.uint32)).view(np.float32)


def build_program():
    nc = bacc.Bacc("TRN2", target_bir_lowering=False, debug=False, num_devices=8)

    def di(name, shape, dt=F32R):
        return nc.dram_tensor(name, shape, dt, kind="ExternalInput").ap()

    xT = di("xT", [HID, S])            # x[b].T
    wqT = di("wqT", [HID, D])          # wq_h.T / 8 (scale folded in)
    wkT = di("wkT", [HID, D])
    wvT = di("wvT", [HID, DV])         # wv_h.T, cols 64..67 zero
    bq = di("bq", [D, 1], F32)         # bq_h / 8
    bk = di("bk", [D, 1], F32)
    bv1 = di("bv1", [DV, 1], F32)      # [bv_h | 1 | 0 0 0] column
    bsynT = di("bsynT", [D, S])        # (onehot @ syn_bias).T
    onehotT = di("onehotT", [D, S])
    woT = di("woT", [D, HID])          # wo[:, hslice].T
    idm = di("idm", [128, 128])        # identity for TensorE transpose
    outT = nc.dram_tensor("outT", [HID, S], F32, kind="ExternalOutput").ap()
    lT = nc.dram_tensor("lT", [1, S], F32, kind="ExternalOutput").ap()

    with tile.TileContext(nc) as tc:
        _body(tc, xT, wqT, wkT, wvT, bq, bk, bv1, bsynT, onehotT, woT, idm,
              outT, lT)
    nc.compile()
    return nc


def _body(tc, xT, wqT, wkT, wvT, bq, bk, bv1, bsynT, onehotT, woT, idm,
          outT, lT):
    nc = tc.nc
    mm = nc.tensor.matmul

    with (
        tc.tile_pool(name="const", bufs=1) as constp,
        tc.tile_pool(name="big", bufs=1) as bigp,
        tc.tile_pool(name="pt", bufs=6) as ptp,
        tc.tile_pool(name="ob", bufs=2) as obp,
    ):
        # ---- constants (DMA'd first so phase A can start immediately) ----
        wq0 = constp.tile([128, D], F32R, name="wq0", tag="wq0")
        wq1 = constp.tile([128, D], F32R, name="wq1", tag="wq1")
        wk0 = constp.tile([128, D], F32R, name="wk0", tag="wk0")
        wk1 = constp.tile([128, D], F32R, name="wk1", tag="wk1")
        wv0 = constp.tile([128, DV], F32R, name="wv0", tag="wv0")
        wv1 = constp.tile([128, DV], F32R, name="wv1", tag="wv1")
        bq_sb = constp.tile([D, 1], F32, name="bq_sb", tag="bq_sb")
        bk_sb = constp.tile([D, 1], F32, name="bk_sb", tag="bk_sb")
        bv1_sb = constp.tile([DV, 1], F32, name="bv1_sb", tag="bv1_sb")
        wo_sb = constp.tile([D, HID], F32R, name="wo_sb", tag="wo_sb")
        id_sb = constp.tile([128, 128], F32R, name="id_sb", tag="id_sb")

        # persistent activations
        xT0 = bigp.tile([128, S], F32R, name="xT0", tag="xT0")
        xT1 = bigp.tile([128, S], F32R, name="xT1", tag="xT1")
        qTt = bigp.tile([128, S], F32R, name="qTt", tag="qTt")  # 0:64 q/8, 64:128 bsynT
        kTt = bigp.tile([128, S], F32R, name="kTt", tag="kTt")  # 0:64 k,   64:128 onehotT
        vTs = bigp.tile([DV, S], F32R, name="vTs", tag="vTs")   # v'^T (d-major)
        vb = bigp.tile([128, NKT * DV], F32R, name="vb", tag="vb")  # v' key-major
        oall = bigp.tile([D, S], F32R, name="oall", tag="oall")   # O^T, unnormalized
        l_sb = bigp.tile([1, S], F32, name="l_sb", tag="l_sb")    # softmax denoms

        # DMA order = need order: x chunk 0 + projection weights, remaining
        # x chunks, then the attention-only tensors (bsynT/onehotT/woT).
        nc.sync.dma_start(xT0[:, 0:QB], xT[0:128, 0:QB])
        nc.sync.dma_start(xT1[:, 0:QB], xT[128:256, 0:QB])
        nc.sync.dma_start(wq0[:], wqT[0:128, :])
        nc.sync.dma_start(wq1[:], wqT[128:256, :])
        nc.sync.dma_start(wk0[:], wkT[0:128, :])
        nc.sync.dma_start(wk1[:], wkT[128:256, :])
        nc.sync.dma_start(bq_sb[:], bq[:])
        nc.sync.dma_start(bk_sb[:], bk[:])
        nc.sync.dma_start(wv0[:], wvT[0:128, :])
        nc.sync.dma_start(wv1[:], wvT[128:256, :])
        nc.sync.dma_start(bv1_sb[:], bv1[:])
        nc.sync.dma_start(id_sb[:], idm[:])
        for c in range(1, NQB):
            cs = slice(c * QB, (c + 1) * QB)
            nc.sync.dma_start(xT0[:, cs], xT[0:128, cs])
            nc.sync.dma_start(xT1[:, cs], xT[128:256, cs])
        nc.sync.dma_start(qTt[64:128, :], bsynT[:])
        nc.sync.dma_start(kTt[64:128, :], onehotT[:])
        nc.sync.dma_start(wo_sb[:], woT[:])

        # ---- phase A: QKV projections, per 512-col chunk as DMA lands ----
        with tc.tile_pool(name="psA", bufs=2, space="PSUM") as psA:
            for t in range(NQB):
                sl = slice(t * QB, (t + 1) * QB)
                qp = psA.tile([D, QB], F32, name="qp", tag="qp")
                mm(qp[:], wq0[:], xT0[:, sl], start=True, stop=False)
                mm(qp[:], wq1[:], xT1[:, sl], start=False, stop=True)
                nc.vector.tensor_scalar_add(qTt[0:D, sl], qp[:], bq_sb[:])

                kp = psA.tile([D, QB], F32, name="kp", tag="kp")
                mm(kp[:], wk0[:], xT0[:, sl], start=True, stop=False)
                mm(kp[:], wk1[:], xT1[:, sl], start=False, stop=True)
                nc.vector.tensor_scalar_add(kTt[0:D, sl], kp[:], bk_sb[:])

                vtp = psA.tile([DV, QB], F32, name="vtp", tag="vtp")
                mm(vtp[:], wv0[:], xT0[:, sl], start=True, stop=False)
                mm(vtp[:], wv1[:], xT1[:, sl], start=False, stop=True)
                # bias column [bv | 1 | 0..]: also creates the ones row
                nc.vector.tensor_scalar_add(vTs[:, sl], vtp[:], bv1_sb[:])

                # flip v' to key-major: 4 TensorE transposes batched into one
                # PSUM tile, single ACT eviction (amortizes the access init)
                vtr = psA.tile([KT, 4 * DV], F32R, name="vtr", tag="vtr")
                for m in range(4):
                    j = 4 * t + m
                    jl = slice(j * KT, (j + 1) * KT)
                    nc.tensor.transpose(vtr[:, m * DV:(m + 1) * DV],
                                        vTs[:, jl], id_sb[0:DV, 0:DV])
                nc.scalar.copy(vb[:, 4 * t * DV:(4 * t + 4) * DV], vtr[:])

        # ---- phase B: flash attention (dense PE stream) ----
        # Software-pipelined emission: the PV matmuls of group g are emitted
        # AFTER the score matmuls of group g+1, so the PE computes the next
        # scores while ACT exponentiates the current group. The output
        # projection of block qb is emitted inside the stream right after
        # its last PV group so output DMA overlaps remaining compute.
        groups = [list(range(g, min(g + GRP, NKT))) for g in range(0, NKT, GRP)]
        with (
            tc.tile_pool(name="psB", bufs=3, space="PSUM") as psB,
            tc.tile_pool(name="psAcc", bufs=2, space="PSUM") as psAcc,
        ):
            oaccs = {}

            def emit_pv(qb, gi, p3):
                qsl = slice(qb * QB, (qb + 1) * QB)
                if gi == 0:
                    oaccs[qb] = psAcc.tile([DV, QB], F32, name="oacc",
                                           tag="oacc")
                oacc = oaccs[qb]
                for m, j in enumerate(groups[gi]):
                    mm(oacc[:], vb[:, j * DV:(j + 1) * DV],
                       p3[:, m * QB:(m + 1) * QB],
                       start=(j == 0), stop=(j == NKT - 1))
                if gi == len(groups) - 1:
                    # stash O^T and l (normalization happens on the host),
                    # then project this block and ship it out
                    nc.vector.tensor_copy(oall[:, qsl], oacc[0:D, :])
                    nc.vector.tensor_copy(l_sb[:, qsl],
                                          oacc[LCOL:LCOL + 1, :])
                    pj = psB.tile([128, 2 * QB], F32, name="pj", tag="s3")
                    mm(pj[:, 0:QB], wo_sb[:, 0:128], oall[:, qsl],
                       start=True, stop=True)
                    mm(pj[:, QB:2 * QB], wo_sb[:, 128:256], oall[:, qsl],
                       start=True, stop=True)
                    ob = obp.tile([128, 2 * QB], F32, name="ob", tag="ob")
                    nc.vector.tensor_copy(ob[:], pj[:])
                    nc.sync.dma_start(outT[0:128, qsl], ob[:, 0:QB])
                    nc.sync.dma_start(outT[128:256, qsl], ob[:, QB:2 * QB])

            pending = None
            for qb in range(NQB):
                qsl = slice(qb * QB, (qb + 1) * QB)
                for gi, js in enumerate(groups):
                    n = len(js)
                    s3 = psB.tile([128, n * QB], F32, name="s3", tag="s3")
                    for m, j in enumerate(js):
                        mm(s3[:, m * QB:(m + 1) * QB],
                           kTt[:, j * KT:(j + 1) * KT], qTt[:, qsl],
                           start=True, stop=True)
                    p3 = ptp.tile([128, n * QB], F32R, name="p3", tag="p3")
                    nc.scalar.activation(p3[:], s3[:], Exp)
                    if pending is not None:
                        emit_pv(*pending)
                    pending = (qb, gi, p3)
            emit_pv(*pending)

            nc.sync.dma_start(lT[:], l_sb[:])


_NC_CACHE = {}


def _get_program():
    if "nc" not in _NC_CACHE:
        _NC_CACHE["nc"] = build_program()
    return _NC_CACHE["nc"]


def make_in_maps(x, codons, syn_bias, wq, bq, wk, bk, wv, bv, wo):
    in_maps = []
    for core in range(8):
        b, h = divmod(core, NH)
        hsl = slice(h * D, (h + 1) * D)
        cod = codons[b]
        onehotT = np.zeros((D, S), np.float32)
        onehotT[cod, np.arange(S)] = 1.0
        in_maps.append({
            "xT": round_fp32r(x[b].T),
            "wqT": round_fp32r(wq[hsl, :].T / 8.0),
            "wkT": round_fp32r(wk[hsl, :].T),
            "wvT": round_fp32r(np.concatenate(
                [wv[hsl, :].T, np.zeros((HID, 4), np.float32)], axis=1)),
            "bq": (bq[hsl] / 8.0).reshape(D, 1).astype(np.float32),
            "bk": bk[hsl].reshape(D, 1).astype(np.float32),
            "bv1": np.concatenate(
                [bv[hsl], [np.float32(1.0)], np.zeros(3, np.float32)]
            ).reshape(DV, 1).astype(np.float32),
            "bsynT": np.ascontiguousarray(syn_bias.T[:, cod]),  # 0/1: f32r-exact
            "onehotT": onehotT,
            "woT": round_fp32r(wo[:, hsl].T),
            "idm": np.eye(128, dtype=np.float32),
        })
    return in_maps


def kernel_run(inputs, trace=False):
    x = np.asarray(inputs["x"], np.float32)
    codons = np.asarray(inputs["codons"]).astype(np.int64)
    syn_bias = np.asarray(inputs["syn_bias"], np.float32)
    wq = np.asarray(inputs["wq"], np.float32)
    bq = np.asarray(inputs["bq"], np.float32)
    wk = np.asarray(inputs["wk"], np.float32)
    bk = np.asarray(inputs["bk"], np.float32)
    wv = np.asarray(inputs["wv"], np.float32)
    bv = np.asarray(inputs["bv"], np.float32)
    wo = np.asarray(inputs["wo"], np.float32)
    bo = np.asarray(inputs["bo"], np.float32)

    nc = _get_program()
    in_maps = make_in_maps(x, codons, syn_bias, wq, bq, wk, bk, wv, bv, wo)
    res = run_bass_kernel_spmd(nc, in_maps, core_ids=list(range(8)), trace=trace)

    out = np.empty((B, S, HID), np.float32)
    for b in range(B):
        acc = None
        for h in range(NH):
            r = res.results[NH * b + h]
            part = r["outT"] / r["lT"]          # normalize per head
            acc = part if acc is None else acc + part
        out[b] = acc.T + bo
    return out, res


def kernel(**inputs):
    out, _ = kernel_run(inputs, trace=False)
    return out



# revision 7
# speedup vs baseline: 1.1964x; 1.1964x over previous
"""CodonAttention Trainium2 kernel (V2: bf16 scores + fp8 PV + dual-engine exp).

Math (per batch b, head h):
  q = x @ wq.T + bq ; k = x @ wk.T + bk ; v = x @ wv.T + bv   (head slices)
  scores = q k^T / 8 + syn_bias[codons_i, codons_j]
  out    = softmax(scores) @ v ;  final = concat_heads(out) @ wo.T + bo

Bias trick: pair_bias factors through one-hots, so augmenting
  q' = [(q+bq)/8 | bsynT] and k' = [k | onehot]  (head dim 128)
gives scores = q'^T k' in one 128-contraction matmul. The softmax
denominator comes free from a ones-column appended to v ([O | l] = P [v | 1]).

V2 speedups over the f32r baseline (196.5us):
- The PV matmul (attn @ v) runs in fp8e4m3 with perf_mode=DoubleRow:
  256 keys of contraction per 512-cycle pass instead of 128 (2x fewer PE
  cycles). p (post-exp weights) and v are quantized to fp8; numerics sim
  puts the end-to-end rel err at ~9e-3 vs the 2e-2 gate.
- exp is split across BOTH elementwise engines: ACT does true exp with
  direct fp8 output; the Vector engine (DVE) produces fp8 weights via a
  Schraudolph-style bit hack -- uint8(round(s * 8/ln2 + C)) IS the e4m3
  bit pattern of ~exp(s) -- one tensor_scalar op per tile. ACT alone
  would be a hard 128us floor; the split brings elementwise to ~85us/engine.
- q/k/x/weights are bf16 (fp8 q/k loses too much: score error 1.5e-2).
  bf16 matmuls run at the same 1 cycle/row as f32r but halve DMA.
All evictions are placed per-engine to balance ACT vs DVE load.

Sharding: 8 cores = (batch b) x (head h). Each core outputs the
unnormalized projected partial outT = (wo_h @ O_h^T) (256, 4096) plus
softmax denominators lT; the host divides, sums heads, transposes, + bo.
"""

import numpy as np
import ml_dtypes

import concourse.mybir as mybir
import concourse.tile as tile
from concourse import bacc
from concourse.bass_utils import run_bass_kernel_spmd


def _ensure_axon_ntff_hook():
    """This image's antenv package lacks axon_hooks, so
    run_bass_kernel_spmd(trace=True) (or BASS_TRACE=1) would die on the
    import. Register a compatible module backed by the libaxon_pjrt C ABI
    so tracing works if a caller requests it."""
    import sys
    try:
        import antenv.axon_hooks  # noqa: F401
        return
    except ImportError:
        pass
    import contextlib
    import ctypes
    import types
    try:
        lib = ctypes.CDLL("/opt/axon/libaxon_pjrt.so")
        has = hasattr(lib, "axon_start_nrt_profile")
    except OSError:
        has = False
    if has:
        lib.axon_start_nrt_profile.argtypes = [ctypes.POINTER(ctypes.c_int64),
                                               ctypes.c_size_t]
        lib.axon_start_nrt_profile.restype = ctypes.c_int64
        lib.axon_stop_nrt_profile.argtypes = [ctypes.c_char_p]
        lib.axon_stop_nrt_profile.restype = ctypes.c_int64

        @contextlib.contextmanager
        def _hook(output_dir, device_ids):
            import jax
            jax.devices()
            if device_ids:
                ids = (ctypes.c_int64 * len(device_ids))(*device_ids)
                rc = lib.axon_start_nrt_profile(ids, len(device_ids))
            else:
                rc = lib.axon_start_nrt_profile(None, 0)
            if rc != 0:
                raise RuntimeError(f"axon_start_nrt_profile rc={rc}")
            try:
                yield
            finally:
                lib.axon_stop_nrt_profile(str(output_dir).encode())
    else:
        _hook = None

    mod = types.ModuleType("antenv.axon_hooks")
    _state = {"hook": _hook}
    mod.get_axon_ntff_profile_hook = lambda: _state["hook"]
    mod.set_axon_ntff_profile_hook = lambda h: _state.__setitem__("hook", h)
    sys.modules["antenv.axon_hooks"] = mod


_ensure_axon_ntff_hook()

B, S, HID, NH, D = 2, 4096, 256, 4, 64
DV = D + 4         # v + ones column + 3 cols padding
QB = 512           # query block (free dim of score matmuls)
KT = 128           # key tile (partition dim of transposed scores)
NQB = S // QB      # 8
NKT = S // KT      # 32
NPAIR = NKT // 2   # 16 exp/PV pair-groups per query block

# Schraudolph fp8 exp: uint8(s * 8/ln2 + C) viewed as e4m3 bits ~= exp(s).
SCH_A = 8.0 / np.log(2.0)
SCH_C = 55.5       # tuned for round-to-nearest float->uint8 conversion

F32 = mybir.dt.float32
F32R = mybir.dt.float32r
BF16 = mybir.dt.bfloat16
FP8 = mybir.dt.float8e4
U8 = mybir.dt.uint8
Exp = mybir.ActivationFunctionType.Exp
DR = mybir.MatmulPerfMode.DoubleRow
MULT = mybir.AluOpType.mult
ADD = mybir.AluOpType.add


def round_fp32r(a):
    """Round-half-up at mantissa bit 12 (walrus fp32_to_fp32r)."""
    a = np.ascontiguousarray(a, np.float32)
    u = a.view(np.uint32).astype(np.uint64)
    return (((u + 0x800) & 0xFFFFF000).astype(np.uint32)).view(np.float32)


def to_bf16(a):
    return np.asarray(a, np.float32).astype(ml_dtypes.bfloat16)


def build_program():
    nc = bacc.Bacc("TRN2", target_bir_lowering=False, debug=False, num_devices=8)

    def di(name, shape, dt):
        return nc.dram_tensor(name, shape, dt, kind="ExternalInput").ap()

    xT = di("xT", [HID, S], BF16)       # x[b].T
    wqT = di("wqT", [HID, D], BF16)     # wq_h.T / 8 (scale folded in)
    wkT = di("wkT", [HID, D], BF16)
    wvT = di("wvT", [HID, DV], BF16)    # wv_h.T, cols 64..67 zero
    bq = di("bq", [D, 1], F32)          # bq_h / 8
    bk = di("bk", [D, 1], F32)
    bv1 = di("bv1", [DV, 1], F32)       # [bv_h | 1 | 0 0 0] column
    bsynT = di("bsynT", [D, S], BF16)   # (onehot @ syn_bias).T
    onehotT = di("onehotT", [D, S], BF16)
    woT = di("woT", [D, HID], F32R)     # wo[:, hslice].T
    idm = di("idm", [128, 128], F32R)   # identity for TensorE transpose
    outT = nc.dram_tensor("outT", [HID, S], F32, kind="ExternalOutput").ap()
    lT = nc.dram_tensor("lT", [1, S], F32, kind="ExternalOutput").ap()

    with tile.TileContext(nc) as tc:
        _body(tc, xT, wqT, wkT, wvT, bq, bk, bv1, bsynT, onehotT, woT, idm,
              outT, lT)
    nc.compile()
    return nc


def _body(tc, xT, wqT, wkT, wvT, bq, bk, bv1, bsynT, onehotT, woT, idm,
          outT, lT):
    nc = tc.nc
    mm = nc.tensor.matmul

    with (
        tc.tile_pool(name="const", bufs=1) as constp,
        tc.tile_pool(name="big", bufs=1) as bigp,
        tc.tile_pool(name="pt", bufs=6) as ptp,
        tc.tile_pool(name="ob", bufs=2) as obp,
    ):
        # ---- constants ----
        wq0 = constp.tile([128, D], BF16, name="wq0", tag="wq0")
        wq1 = constp.tile([128, D], BF16, name="wq1", tag="wq1")
        wk0 = constp.tile([128, D], BF16, name="wk0", tag="wk0")
        wk1 = constp.tile([128, D], BF16, name="wk1", tag="wk1")
        wv0 = constp.tile([128, DV], BF16, name="wv0", tag="wv0")
        wv1 = constp.tile([128, DV], BF16, name="wv1", tag="wv1")
        bq_sb = constp.tile([D, 1], F32, name="bq_sb", tag="bq_sb")
        bk_sb = constp.tile([D, 1], F32, name="bk_sb", tag="bk_sb")
        bv1_sb = constp.tile([DV, 1], F32, name="bv1_sb", tag="bv1_sb")
        wo_sb = constp.tile([D, HID], F32R, name="wo_sb", tag="wo_sb")
        id_sb = constp.tile([128, 128], F32R, name="id_sb", tag="id_sb")

        # persistent activations
        xT0 = bigp.tile([128, S], BF16, name="xT0", tag="xT0")
        xT1 = bigp.tile([128, S], BF16, name="xT1", tag="xT1")
        qTt = bigp.tile([128, S], BF16, name="qTt", tag="qTt")  # 0:64 q/8, 64:128 bsynT
        kTt = bigp.tile([128, S], BF16, name="kTt", tag="kTt")  # 0:64 k,   64:128 onehotT
        vTs = bigp.tile([DV, S], F32R, name="vTs", tag="vTs")   # v'^T (d-major)
        # v' key-major; slot padded to 128 cols: the DoubleRow ldweights ISA
        # check needs step_elem%16==0 and col_grp=0xf (full 128 array cols).
        vb = bigp.tile([128, NKT, 128], FP8, name="vb", tag="vb")
        oall = bigp.tile([D + 1, S], F32R, name="oall", tag="oall")  # [O^T | l]

        # DMA order = need order: x chunks + weights, then k-side bias
        # tensors (scores need kTt complete first), then the rest.
        nc.sync.dma_start(xT0[:, 0:QB], xT[0:128, 0:QB])
        nc.sync.dma_start(xT1[:, 0:QB], xT[128:256, 0:QB])
        nc.sync.dma_start(wk0[:], wkT[0:128, :])
        nc.sync.dma_start(wk1[:], wkT[128:256, :])
        nc.sync.dma_start(bk_sb[:], bk[:])
        nc.sync.dma_start(wq0[:], wqT[0:128, :])
        nc.sync.dma_start(wq1[:], wqT[128:256, :])
        nc.sync.dma_start(bq_sb[:], bq[:])
        nc.sync.dma_start(wv0[:], wvT[0:128, :])
        nc.sync.dma_start(wv1[:], wvT[128:256, :])
        nc.sync.dma_start(bv1_sb[:], bv1[:])
        nc.sync.dma_start(id_sb[:], idm[:])
        for c in range(1, NQB):
            cs = slice(c * QB, (c + 1) * QB)
            nc.sync.dma_start(xT0[:, cs], xT[0:128, cs])
            nc.sync.dma_start(xT1[:, cs], xT[128:256, cs])
        nc.sync.dma_start(kTt[64:128, :], onehotT[:])
        nc.sync.dma_start(qTt[64:128, :], bsynT[:])
        nc.sync.dma_start(wo_sb[:], woT[:])

        # zero vb's padding columns (GPSIMD: idle engine, runs under phase A)
        nc.gpsimd.memset(vb[:, :, :], 0.0)

        # ---- phase A: QKV projections (k first so scores can start) ----
        with tc.tile_pool(name="psA", bufs=2, space="PSUM") as psA:
            for t in range(NQB):
                sl = slice(t * QB, (t + 1) * QB)
                kp = psA.tile([D, QB], F32, name="kp", tag="kp")
                mm(kp[:], wk0[:], xT0[:, sl], start=True, stop=False)
                mm(kp[:], wk1[:], xT1[:, sl], start=False, stop=True)
                nc.scalar.add(kTt[0:D, sl], kp[:], bk_sb[:])

            for t in range(NQB):
                sl = slice(t * QB, (t + 1) * QB)
                qp = psA.tile([D, QB], F32, name="qp", tag="qp")
                mm(qp[:], wq0[:], xT0[:, sl], start=True, stop=False)
                mm(qp[:], wq1[:], xT1[:, sl], start=False, stop=True)
                nc.scalar.add(qTt[0:D, sl], qp[:], bq_sb[:])

            for t in range(NQB):
                sl = slice(t * QB, (t + 1) * QB)
                vtp = psA.tile([DV, QB], F32, name="vtp", tag="vtp")
                mm(vtp[:], wv0[:], xT0[:, sl], start=True, stop=False)
                mm(vtp[:], wv1[:], xT1[:, sl], start=False, stop=True)
                # bias column [bv | 1 | 0..] also creates the ones row
                nc.scalar.add(vTs[:, sl], vtp[:], bv1_sb[:])

                # flip v' to key-major: 4 TensorE transposes into one PSUM
                # tile, single fp8-converting eviction
                vtr = psA.tile([KT, 4, DV], F32R, name="vtr", tag="vtr")
                for m in range(4):
                    j = 4 * t + m
                    jl = slice(j * KT, (j + 1) * KT)
                    nc.tensor.transpose(vtr[:, m, :], vTs[:, jl],
                                        id_sb[0:DV, 0:DV])
                nc.scalar.copy(vb[:, 4 * t:4 * t + 4, 0:DV], vtr[:, :, :])

        # ---- phase B: flash attention ----
        # Pair-groups of 2 key tiles: scores land in a 2-bank PSUM tile,
        # one exp op covers both, and the PV matmul contracts both key
        # tiles at once via fp8 DoubleRow. exp alternates ACT / DVE.
        # Software-pipelined: PV of group g is emitted after the score
        # matmuls of group g+1.
        with (
            tc.tile_pool(name="psB", bufs=3, space="PSUM") as psB,
            tc.tile_pool(name="psAcc", bufs=2, space="PSUM") as psAcc,
        ):
            oaccs = {}

            def emit_pv(qb, g, p3):
                qsl = slice(qb * QB, (qb + 1) * QB)
                if g == 0:
                    oaccs[qb] = psAcc.tile([128, QB], F32, name="oacc",
                                           tag="oacc")
                oacc = oaccs[qb]
                mm(oacc[:], vb[:, 2 * g:2 * g + 2, :], p3[:, :, :],
                   start=(g == 0), stop=(g == NPAIR - 1), perf_mode=DR)
                if g == NPAIR - 1:
                    # stash [O^T | l] (normalization happens on the host),
                    # then project this block and ship it out
                    nc.scalar.copy(oall[:, qsl], oacc[0:D + 1, :])
                    pj = psB.tile([128, 2, QB], F32, name="pj", tag="s3")
                    mm(pj[:, 0, :], wo_sb[:, 0:128], oall[0:D, qsl],
                       start=True, stop=True)
                    mm(pj[:, 1, :], wo_sb[:, 128:256], oall[0:D, qsl],
                       start=True, stop=True)
                    ob = obp.tile([128, 2, QB], F32, name="ob", tag="ob")
                    nc.vector.tensor_copy(ob[:, :, :], pj[:, :, :])
                    nc.sync.dma_start(outT[0:128, qsl], ob[:, 0, :])
                    nc.sync.dma_start(outT[128:256, qsl], ob[:, 1, :])

            pending = None
            for qb in range(NQB):
                qsl = slice(qb * QB, (qb + 1) * QB)
                for g in range(NPAIR):
                    s3 = psB.tile([128, 2, QB], F32, name="s3", tag="s3")
                    for i in (0, 1):
                        j = 2 * g + i
                        mm(s3[:, i, :], kTt[:, j * KT:(j + 1) * KT],
                           qTt[:, qsl], start=True, stop=True)
                    p3 = ptp.tile([128, 2, QB], FP8, name="p3", tag="p3")
                    if (qb * NPAIR + g) % 2 == 0:
                        nc.scalar.activation(p3[:, :, :], s3[:, :, :], Exp)
                    else:
                        nc.vector.tensor_scalar(
                            p3[:, :, :].bitcast(U8), s3[:, :, :],
                            float(SCH_A), float(SCH_C), MULT, ADD)
                    if pending is not None:
                        emit_pv(*pending)
                    pending = (qb, g, p3)
            emit_pv(*pending)

            nc.sync.dma_start(lT[:], oall[D:D + 1, :].bitcast(F32))


_NC_CACHE = {}


def _get_program():
    if "nc" not in _NC_CACHE:
        _NC_CACHE["nc"] = build_program()
    return _NC_CACHE["nc"]


def make_in_maps(x, codons, syn_bias, wq, bq, wk, bk, wv, bv, wo):
    in_maps = []
    for core in range(8):
        b, h = divmod(core, NH)
        hsl = slice(h * D, (h + 1) * D)
        cod = codons[b]
        onehotT = np.zeros((D, S), np.float32)
        onehotT[cod, np.arange(S)] = 1.0
        in_maps.append({
            "xT": to_bf16(x[b].T),
            "wqT": to_bf16(wq[hsl, :].T / 8.0),
            "wkT": to_bf16(wk[hsl, :].T),
            "wvT": to_bf16(np.concatenate(
                [wv[hsl, :].T, np.zeros((HID, 4), np.float32)], axis=1)),
            "bq": (bq[hsl] / 8.0).reshape(D, 1).astype(np.float32),
            "bk": bk[hsl].reshape(D, 1).astype(np.float32),
            "bv1": np.concatenate(
                [bv[hsl], [np.float32(1.0)], np.zeros(3, np.float32)]
            ).reshape(DV, 1).astype(np.float32),
            "bsynT": to_bf16(syn_bias.T[:, cod]),
            "onehotT": to_bf16(onehotT),
            "woT": round_fp32r(wo[:, hsl].T),
            "idm": np.eye(128, dtype=np.float32),
        })
    return in_maps


def kernel_run(inputs, trace=False):
    x = np.asarray(inputs["x"], np.float32)
    codons = np.asarray(inputs["codons"]).astype(np.int64)
    syn_bias = np.asarray(inputs["syn_bias"], np.float32)
    wq = np.asarray(inputs["wq"], np.float32)
    bq = np.asarray(inputs["bq"], np.float32)
    wk = np.asarray(inputs["wk"], np.float32)
    bk = np.asarray(inputs["bk"], np.float32)
    wv = np.asarray(inputs["wv"], np.float32)
    bv = np.asarray(inputs["bv"], np.float32)
    wo = np.asarray(inputs["wo"], np.float32)
    bo = np.asarray(inputs["bo"], np.float32)

    nc = _get_program()
    in_maps = make_in_maps(x, codons, syn_bias, wq, bq, wk, bk, wv, bv, wo)
    res = run_bass_kernel_spmd(nc, in_maps, core_ids=list(range(8)), trace=trace)

    out = np.empty((B, S, HID), np.float32)
    for b in range(B):
        acc = None
        for h in range(NH):
            r = res.results[NH * b + h]
            part = r["outT"] / r["lT"]          # normalize per head
            acc = part if acc is None else acc + part
        out[b] = acc.T + bo
    return out, res


def kernel(**inputs):
    out, _ = kernel_run(inputs, trace=False)
    return out


# revision 9
# speedup vs baseline: 1.2496x; 1.0445x over previous
"""CodonAttention Trainium2 kernel (V3: bf16 scores + fp8 PV + dual-engine exp).

Math (per batch b, head h):
  q = x @ wq.T + bq ; k = x @ wk.T + bk ; v = x @ wv.T + bv   (head slices)
  scores = q k^T / 8 + syn_bias[codons_i, codons_j]
  out    = softmax(scores) @ v ;  final = concat_heads(out) @ wo.T + bo

Bias trick: pair_bias factors through one-hots, so augmenting
  q' = [(q+bq)/8 | bsynT] and k' = [k | onehot]  (head dim 128)
gives scores = q'^T k' in one 128-contraction matmul. The softmax
denominator comes free from a ones-column appended to v ([O | l] = P [v | 1]).

Speedups over the f32r baseline (196.5us):
- PV matmul (attn @ v) in fp8e4m3 with perf_mode=DoubleRow: 256 keys of
  contraction per 512-cycle pass (2x fewer PE cycles). The v stationary
  slots are padded to 128 cols (DoubleRow ISA wants col_grp=0xf and
  16B-aligned k-pair stride).
- exp split across BOTH elementwise engines: ACT does true exp with fp8
  output; DVE makes fp8 weights via a Schraudolph bit hack --
  uint8(s * 8/ln2 + C) IS the e4m3 bit pattern of ~exp(s) -- one
  tensor_scalar per tile. ACT alone would be a 128us floor.
- q/k/x/weights bf16 (fp8 q/k costs 1.5e-2 rel err -- too much). bf16
  matmuls run at 1 cycle/row like f32r but halve DMA.
- Few, fat DMAs: the Sync engine serializes dma_start triggers at
  ~650ns each, so inputs are packed into single 3D transfers.
- Phase-A projections load each stationary once per TWO chunks and
  evictions alternate ACT/DVE so neither engine paces the PE.
- Phase-B software pipeline depth 2: PV(g) is emitted after scores(g+2),
  hiding the ~1.2us exp latency behind two score pairs.

Sharding: 8 cores = (batch b) x (head h). Each core outputs the
unnormalized projected partial outT = (wo_h @ O_h^T) (256, 4096) plus
softmax denominators lT; the host divides, sums heads, transposes, + bo.
"""

import numpy as np
import ml_dtypes

import concourse.mybir as mybir
import concourse.tile as tile
from concourse import bacc
from concourse.bass_utils import run_bass_kernel_spmd


def _ensure_axon_ntff_hook():
    """This image's antenv package lacks axon_hooks; recreate it from the
    libaxon_pjrt C ABI so run_bass_kernel_spmd(trace=True) works."""
    import sys
    try:
        import antenv.axon_hooks  # noqa: F401
        return
    except ImportError:
        pass
    import contextlib
    import ctypes
    import types
    try:
        lib = ctypes.CDLL("/opt/axon/libaxon_pjrt.so")
        has = hasattr(lib, "axon_start_nrt_profile")
    except OSError:
        has = False
    if has:
        lib.axon_start_nrt_profile.argtypes = [ctypes.POINTER(ctypes.c_int64),
                                               ctypes.c_size_t]
        lib.axon_start_nrt_profile.restype = ctypes.c_int64
        lib.axon_stop_nrt_profile.argtypes = [ctypes.c_char_p]
        lib.axon_stop_nrt_profile.restype = ctypes.c_int64

        @contextlib.contextmanager
        def _hook(output_dir, device_ids):
            import jax
            jax.devices()
            if device_ids:
                ids = (ctypes.c_int64 * len(device_ids))(*device_ids)
                rc = lib.axon_start_nrt_profile(ids, len(device_ids))
            else:
                rc = lib.axon_start_nrt_profile(None, 0)
            if rc != 0:
                raise RuntimeError(f"axon_start_nrt_profile rc={rc}")
            try:
                yield
            finally:
                lib.axon_stop_nrt_profile(str(output_dir).encode())
    else:
        _hook = None

    mod = types.ModuleType("antenv.axon_hooks")
    _state = {"hook": _hook}
    mod.get_axon_ntff_profile_hook = lambda: _state["hook"]
    mod.set_axon_ntff_profile_hook = lambda h: _state.__setitem__("hook", h)
    sys.modules["antenv.axon_hooks"] = mod


_ensure_axon_ntff_hook()

B, S, HID, NH, D = 2, 4096, 256, 4, 64
DV = D + 4         # v + ones column + 3 cols padding
VBW = 128          # vb key-tile slot width (DoubleRow ldweights: col_grp=0xf)
QB = 512           # query block (free dim of score matmuls)
KT = 128           # key tile (partition dim of transposed scores)
NQB = S // QB      # 8
NKT = S // KT      # 32
NPAIR = NKT // 2   # 16 exp/PV pair-groups per query block
PIPE = 2           # PV lags the score stream by this many pair-groups

# Schraudolph fp8 exp: uint8(s * 8/ln2 + C) viewed as e4m3 bits ~= exp(s).
SCH_A = 8.0 / np.log(2.0)
SCH_C = 55.5       # tuned for round-to-nearest float->uint8 conversion

F32 = mybir.dt.float32
F32R = mybir.dt.float32r
BF16 = mybir.dt.bfloat16
FP8 = mybir.dt.float8e4
U8 = mybir.dt.uint8
Exp = mybir.ActivationFunctionType.Exp
DR = mybir.MatmulPerfMode.DoubleRow
MULT = mybir.AluOpType.mult
ADD = mybir.AluOpType.add


def round_fp32r(a):
    """Round-half-up at mantissa bit 12 (walrus fp32_to_fp32r)."""
    a = np.ascontiguousarray(a, np.float32)
    u = a.view(np.uint32).astype(np.uint64)
    return (((u + 0x800) & 0xFFFFF000).astype(np.uint32)).view(np.float32)


def to_bf16(a):
    return np.asarray(a, np.float32).astype(ml_dtypes.bfloat16)


def split_hid(a):
    """(256, N) -> (128, 2, N): row blocks side by side per partition."""
    a = np.ascontiguousarray(a)
    return np.ascontiguousarray(a.reshape(2, 128, a.shape[1]).transpose(1, 0, 2))


def build_program():
    nc = bacc.Bacc("TRN2", target_bir_lowering=False, debug=False, num_devices=8)

    def di(name, shape, dt):
        return nc.dram_tensor(name, shape, dt, kind="ExternalInput").ap()

    xT = di("xT", [128, 2, S], BF16)    # x[b].T hidden-split
    wq2 = di("wq2", [128, 2, D], BF16)  # wq_h.T / 8 hidden-split
    wk2 = di("wk2", [128, 2, D], BF16)
    wv2 = di("wv2", [128, 2, DV], BF16)  # col 64 zero (ones col via bias)
    bias3 = di("bias3", [DV, 3], F32)   # [bq/8 | bk | bv1] columns
    bsynT = di("bsynT", [D, S], BF16)   # (onehot @ syn_bias).T
    onehotT = di("onehotT", [D, S], BF16)
    woT = di("woT", [D, HID], F32R)     # wo[:, hslice].T
    idm = di("idm", [128, 128], F32R)   # identity for TensorE transpose
    outT = nc.dram_tensor("outT", [HID, S], F32, kind="ExternalOutput").ap()
    lT = nc.dram_tensor("lT", [1, S], F32, kind="ExternalOutput").ap()

    with tile.TileContext(nc) as tc:
        _body(tc, xT, wq2, wk2, wv2, bias3, bsynT, onehotT, woT, idm,
              outT, lT)
    nc.compile()
    return nc


def _body(tc, xT, wq2, wk2, wv2, bias3, bsynT, onehotT, woT, idm, outT, lT):
    nc = tc.nc
    mm = nc.tensor.matmul

    with (
        tc.tile_pool(name="const", bufs=1) as constp,
        tc.tile_pool(name="big", bufs=1) as bigp,
        tc.tile_pool(name="pt", bufs=6) as ptp,
        tc.tile_pool(name="ob", bufs=2) as obp,
    ):
        # ---- constants ----
        wq_sb = constp.tile([128, 2, D], BF16, name="wq_sb", tag="wq_sb")
        wk_sb = constp.tile([128, 2, D], BF16, name="wk_sb", tag="wk_sb")
        wv_sb = constp.tile([128, 2, DV], BF16, name="wv_sb", tag="wv_sb")
        b3_sb = constp.tile([DV, 3], F32, name="b3_sb", tag="b3_sb")
        wo_sb = constp.tile([D, HID], F32R, name="wo_sb", tag="wo_sb")
        id_sb = constp.tile([128, 128], F32R, name="id_sb", tag="id_sb")

        # persistent activations
        xTa = bigp.tile([128, 2, S], BF16, name="xTa", tag="xTa")
        qTt = bigp.tile([128, S], BF16, name="qTt", tag="qTt")  # 0:64 q/8, 64:128 bsynT
        kTt = bigp.tile([128, S], BF16, name="kTt", tag="kTt")  # 0:64 k,   64:128 onehotT
        vTs = bigp.tile([DV, S], F32R, name="vTs", tag="vTs")   # v'^T (d-major)
        vb = bigp.tile([128, NKT, VBW], FP8, name="vb", tag="vb")  # v' key-major
        oall = bigp.tile([D + 1, S], F32R, name="oall", tag="oall")  # [O^T | l]

        # Few fat DMAs (Sync issues triggers at ~650ns each, so batch).
        # Order = need order: k weights, x head chunk, rest of x, the rest.
        nc.sync.dma_start(wk_sb[:], wk2[:])
        nc.sync.dma_start(b3_sb[:], bias3[:])
        nc.sync.dma_start(xTa[:, :, 0:2 * QB], xT[:, :, 0:2 * QB])
        nc.sync.dma_start(xTa[:, :, 2 * QB:S], xT[:, :, 2 * QB:S])
        nc.sync.dma_start(wq_sb[:], wq2[:])
        nc.sync.dma_start(wv_sb[:], wv2[:])
        nc.sync.dma_start(id_sb[:], idm[:])
        nc.sync.dma_start(kTt[64:128, :], onehotT[:])
        nc.sync.dma_start(qTt[64:128, :], bsynT[:])
        nc.sync.dma_start(wo_sb[:], woT[:])

        # zero vb's padding columns (GPSIMD: idle engine, runs under phase A)
        nc.gpsimd.memset(vb[:, :, :], 0.0)

        bq_ap = b3_sb[0:D, 0:1]
        bk_ap = b3_sb[0:D, 1:2]
        bv1_ap = b3_sb[0:DV, 2:3]

        # ---- phase A: QKV projections ----
        # K first (scores need all of kTt), each stationary loaded once per
        # two chunks, evictions split DVE (k,q) / ACT (v,vb).
        with tc.tile_pool(name="psProj", bufs=6, space="PSUM") as psP, \
             tc.tile_pool(name="psTr", bufs=2, space="PSUM") as psT:

            def proj_pair(w_sb, p, width):
                ps = []
                for t in (2 * p, 2 * p + 1):
                    sl = slice(t * QB, (t + 1) * QB)
                    pp = psP.tile([DV, QB], F32, name="pp", tag="pp")
                    mm(pp[0:width, :], w_sb[:, 0, 0:width], xTa[:, 0, sl],
                       start=True, stop=False)
                    ps.append((pp, sl))
                for pp, sl in ps:
                    mm(pp[0:width, :], w_sb[:, 1, 0:width], xTa[:, 1, sl],
                       start=False, stop=True)
                return ps

            for p in range(NQB // 2):
                for pp, sl in proj_pair(wk_sb, p, D):
                    nc.vector.tensor_scalar_add(kTt[0:D, sl], pp[0:D, :], bk_ap)
            for p in range(NQB // 2):
                for pp, sl in proj_pair(wq_sb, p, D):
                    nc.vector.tensor_scalar_add(qTt[0:D, sl], pp[0:D, :], bq_ap)
            for p in range(NQB // 2):
                pairs = proj_pair(wv_sb, p, DV)
                for pi, (pp, sl) in enumerate(pairs):
                    t = 2 * p + pi
                    # bias column [bv | 1 | 0..] also creates the ones row
                    nc.scalar.add(vTs[:, sl], pp[:], bv1_ap)
                    # flip v' to key-major: 4 TensorE transposes into one
                    # PSUM tile, single fp8-converting eviction
                    vtr = psT.tile([KT, 4, DV], F32R, name="vtr", tag="vtr")
                    for m in range(4):
                        j = 4 * t + m
                        jl = slice(j * KT, (j + 1) * KT)
                        nc.tensor.transpose(vtr[:, m, :], vTs[:, jl],
                                            id_sb[0:DV, 0:DV])
                    nc.scalar.copy(vb[:, 4 * t:4 * t + 4, 0:DV], vtr[:, :, :])

        # ---- phase B: flash attention ----
        # Pair-groups of 2 key tiles: scores land in a 2-bank PSUM tile,
        # one exp op covers both, and the PV matmul contracts both key
        # tiles at once via fp8 DoubleRow. exp alternates ACT / DVE.
        # Software pipeline: PV of group g runs after scores of g+PIPE.
        with (
            tc.tile_pool(name="psB", bufs=3, space="PSUM") as psB,
            tc.tile_pool(name="psAcc", bufs=2, space="PSUM") as psAcc,
        ):
            oaccs = {}

            def emit_pv(qb, g, p3):
                qsl = slice(qb * QB, (qb + 1) * QB)
                if g == 0:
                    oaccs[qb] = psAcc.tile([128, QB], F32, name="oacc",
                                           tag="oacc")
                oacc = oaccs[qb]
                mm(oacc[:], vb[:, 2 * g:2 * g + 2, :], p3[:, :, :],
                   start=(g == 0), stop=(g == NPAIR - 1), perf_mode=DR)
                if g == NPAIR - 1:
                    # stash [O^T | l] (normalization happens on the host),
                    # then project this block and ship it out
                    nc.scalar.copy(oall[:, qsl], oacc[0:D + 1, :])
                    pj = psB.tile([128, 2, QB], F32, name="pj", tag="s3")
                    mm(pj[:, 0, :], wo_sb[:, 0:128], oall[0:D, qsl],
                       start=True, stop=True)
                    mm(pj[:, 1, :], wo_sb[:, 128:256], oall[0:D, qsl],
                       start=True, stop=True)
                    ob = obp.tile([128, 2, QB], F32, name="ob", tag="ob")
                    nc.scalar.copy(ob[:, :, :], pj[:, :, :])
                    nc.sync.dma_start(outT[0:128, qsl], ob[:, 0, :])
                    nc.sync.dma_start(outT[128:256, qsl], ob[:, 1, :])

            pending = []
            for qb in range(NQB):
                qsl = slice(qb * QB, (qb + 1) * QB)
                for g in range(NPAIR):
                    s3 = psB.tile([128, 2, QB], F32, name="s3", tag="s3")
                    for i in (0, 1):
                        j = 2 * g + i
                        mm(s3[:, i, :], kTt[:, j * KT:(j + 1) * KT],
                           qTt[:, qsl], start=True, stop=True)
                    p3 = ptp.tile([128, 2, QB], FP8, name="p3", tag="p3")
                    if (qb * NPAIR + g) % 2 == 0:
                        nc.scalar.activation(p3[:, :, :], s3[:, :, :], Exp)
                    else:
                        nc.vector.tensor_scalar(
                            p3[:, :, :].bitcast(U8), s3[:, :, :],
                            float(SCH_A), float(SCH_C), MULT, ADD)
                    pending.append((qb, g, p3))
                    if len(pending) > PIPE:
                        emit_pv(*pending.pop(0))
            while pending:
                emit_pv(*pending.pop(0))

            nc.sync.dma_start(lT[:], oall[D:D + 1, :].bitcast(F32))


_NC_CACHE = {}


def _get_program():
    if "nc" not in _NC_CACHE:
        _NC_CACHE["nc"] = build_program()
    return _NC_CACHE["nc"]


def make_in_maps(x, codons, syn_bias, wq, bq, wk, bk, wv, bv, wo):
    in_maps = []
    for core in range(8):
        b, h = divmod(core, NH)
        hsl = slice(h * D, (h + 1) * D)
        cod = codons[b]
        onehotT = np.zeros((D, S), np.float32)
        onehotT[cod, np.arange(S)] = 1.0
        bias3 = np.zeros((DV, 3), np.float32)
        bias3[0:D, 0] = bq[hsl] / 8.0
        bias3[0:D, 1] = bk[hsl]
        bias3[0:D, 2] = bv[hsl]
        bias3[D, 2] = 1.0
        in_maps.append({
            "xT": split_hid(to_bf16(x[b].T)),
            "wq2": split_hid(to_bf16(wq[hsl, :].T / 8.0)),
            "wk2": split_hid(to_bf16(wk[hsl, :].T)),
            "wv2": split_hid(to_bf16(np.concatenate(
                [wv[hsl, :].T, np.zeros((HID, 4), np.float32)], axis=1))),
            "bias3": bias3,
            "bsynT": to_bf16(syn_bias.T[:, cod]),
            "onehotT": to_bf16(onehotT),
            "woT": round_fp32r(wo[:, hsl].T),
            "idm": np.eye(128, dtype=np.float32),
        })
    return in_maps


def kernel_run(inputs, trace=False):
    x = np.asarray(inputs["x"], np.float32)
    codons = np.asarray(inputs["codons"]).astype(np.int64)
    syn_bias = np.asarray(inputs["syn_bias"], np.float32)
    wq = np.asarray(inputs["wq"], np.float32)
    bq = np.asarray(inputs["bq"], np.float32)
    wk = np.asarray(inputs["wk"], np.float32)
    bk = np.asarray(inputs["bk"], np.float32)
    wv = np.asarray(inputs["wv"], np.float32)
    bv = np.asarray(inputs["bv"], np.float32)
    wo = np.asarray(inputs["wo"], np.float32)
    bo = np.asarray(inputs["bo"], np.float32)

    nc = _get_program()
    in_maps = make_in_maps(x, codons, syn_bias, wq, bq, wk, bk, wv, bv, wo)
    res = run_bass_kernel_spmd(nc, in_maps, core_ids=list(range(8)), trace=trace)

    out = np.empty((B, S, HID), np.float32)
    for b in range(B):
        acc = None
        for h in range(NH):
            r = res.results[NH * b + h]
            part = r["outT"] / r["lT"]          # normalize per head
            acc = part if acc is None else acc + part
        out[b] = acc.T + bo
    return out, res


def kernel(**inputs):
    out, _ = kernel_run(inputs, trace=False)
    return out


# revision 12
# speedup vs baseline: 1.3613x; 1.0894x over previous
"""CodonAttention Trainium2 kernel (V3: bf16 scores + fp8 PV + dual-engine exp).

Math (per batch b, head h):
  q = x @ wq.T + bq ; k = x @ wk.T + bk ; v = x @ wv.T + bv   (head slices)
  scores = q k^T / 8 + syn_bias[codons_i, codons_j]
  out    = softmax(scores) @ v ;  final = concat_heads(out) @ wo.T + bo

Bias trick: pair_bias factors through one-hots, so augmenting
  q' = [(q+bq)/8 | bsynT] and k' = [k | onehot]  (head dim 128)
gives scores = q'^T k' in one 128-contraction matmul. The softmax
denominator comes free from a ones-column appended to v ([O | l] = P [v | 1]).

Speedups over the f32r baseline (196.5us):
- PV matmul (attn @ v) in fp8e4m3 with perf_mode=DoubleRow: 256 keys of
  contraction per 512-cycle pass (2x fewer PE cycles). The v stationary
  slots are padded to 128 cols (DoubleRow ISA wants col_grp=0xf and
  16B-aligned k-pair stride).
- exp split across BOTH elementwise engines: ACT does true exp with fp8
  output; DVE makes fp8 weights via a Schraudolph bit hack --
  uint8(s * 8/ln2 + C) IS the e4m3 bit pattern of ~exp(s) -- one
  tensor_scalar per tile. ACT alone would be a 128us floor.
- q/k/x/weights bf16 (fp8 q/k costs 1.5e-2 rel err -- too much). bf16
  matmuls run at 1 cycle/row like f32r but halve DMA.
- Few, fat DMAs: the Sync engine serializes dma_start triggers at
  ~650ns each, so inputs are packed into single 3D transfers.
- Phase-A projections load each stationary once per TWO chunks and
  evictions alternate ACT/DVE so neither engine paces the PE.
- Phase-B software pipeline depth 2: PV(g) is emitted after scores(g+2),
  hiding the ~1.2us exp latency behind two score pairs.

Sharding: 8 cores = (batch b) x (head h). Each core outputs the
unnormalized projected partial outT = (wo_h @ O_h^T) (256, 4096) plus
softmax denominators lT; the host divides, sums heads, transposes, + bo.
"""

import numpy as np
import ml_dtypes

import concourse.mybir as mybir
import concourse.tile as tile
from concourse import bacc
from concourse.bass_utils import run_bass_kernel_spmd


def _ensure_axon_ntff_hook():
    """This image's antenv package lacks axon_hooks; recreate it from the
    libaxon_pjrt C ABI so run_bass_kernel_spmd(trace=True) works."""
    import sys
    try:
        import antenv.axon_hooks  # noqa: F401
        return
    except ImportError:
        pass
    import contextlib
    import ctypes
    import types
    try:
        lib = ctypes.CDLL("/opt/axon/libaxon_pjrt.so")
        has = hasattr(lib, "axon_start_nrt_profile")
    except OSError:
        has = False
    if has:
        lib.axon_start_nrt_profile.argtypes = [ctypes.POINTER(ctypes.c_int64),
                                               ctypes.c_size_t]
        lib.axon_start_nrt_profile.restype = ctypes.c_int64
        lib.axon_stop_nrt_profile.argtypes = [ctypes.c_char_p]
        lib.axon_stop_nrt_profile.restype = ctypes.c_int64

        @contextlib.contextmanager
        def _hook(output_dir, device_ids):
            import jax
            jax.devices()
            if device_ids:
                ids = (ctypes.c_int64 * len(device_ids))(*device_ids)
                rc = lib.axon_start_nrt_profile(ids, len(device_ids))
            else:
                rc = lib.axon_start_nrt_profile(None, 0)
            if rc != 0:
                raise RuntimeError(f"axon_start_nrt_profile rc={rc}")
            try:
                yield
            finally:
                lib.axon_stop_nrt_profile(str(output_dir).encode())
    else:
        _hook = None

    mod = types.ModuleType("antenv.axon_hooks")
    _state = {"hook": _hook}
    mod.get_axon_ntff_profile_hook = lambda: _state["hook"]
    mod.set_axon_ntff_profile_hook = lambda h: _state.__setitem__("hook", h)
    sys.modules["antenv.axon_hooks"] = mod


_ensure_axon_ntff_hook()

B, S, HID, NH, D = 2, 4096, 256, 4, 64
DV = D + 4         # v + ones column + 3 cols padding
VBW = 128          # vb key-tile slot width (DoubleRow ldweights: col_grp=0xf)
QB = 512           # query block (free dim of score matmuls)
KT = 128           # key tile (partition dim of transposed scores)
NQB = S // QB      # 8
NKT = S // KT      # 32
NPAIR = NKT // 2   # 16 exp/PV pair-groups per query block
PIPE = 2           # PV lags the score stream by this many pair-groups

# Schraudolph fp8 exp: uint8(s * 8/ln2 + C) viewed as e4m3 bits ~= exp(s).
SCH_A = 8.0 / np.log(2.0)
SCH_C = 55.5       # tuned for round-to-nearest float->uint8 conversion

F32 = mybir.dt.float32
F32R = mybir.dt.float32r
BF16 = mybir.dt.bfloat16
FP8 = mybir.dt.float8e4
U8 = mybir.dt.uint8
Exp = mybir.ActivationFunctionType.Exp
DR = mybir.MatmulPerfMode.DoubleRow
MULT = mybir.AluOpType.mult
ADD = mybir.AluOpType.add


def round_fp32r(a):
    """Round-half-up at mantissa bit 12 (walrus fp32_to_fp32r)."""
    a = np.ascontiguousarray(a, np.float32)
    u = a.view(np.uint32).astype(np.uint64)
    return (((u + 0x800) & 0xFFFFF000).astype(np.uint32)).view(np.float32)


def to_bf16(a):
    return np.asarray(a, np.float32).astype(ml_dtypes.bfloat16)


def split_hid(a):
    """(256, N) -> (128, 2, N): row blocks side by side per partition."""
    a = np.ascontiguousarray(a)
    return np.ascontiguousarray(a.reshape(2, 128, a.shape[1]).transpose(1, 0, 2))


def build_program():
    nc = bacc.Bacc("TRN2", target_bir_lowering=False, debug=False, num_devices=8)

    def di(name, shape, dt):
        return nc.dram_tensor(name, shape, dt, kind="ExternalInput").ap()

    xT = di("xT", [128, 2, S], BF16)    # x[b].T hidden-split
    wq2 = di("wq2", [128, 2, D], BF16)  # wq_h.T / 8 hidden-split
    wk2 = di("wk2", [128, 2, D], BF16)
    wv2 = di("wv2", [128, 2, DV], BF16)  # col 64 zero (ones col via bias)
    bias3 = di("bias3", [DV, 3], F32)   # [bq/8 | bk | bv1] columns
    bsynT = di("bsynT", [D, S], BF16)   # (onehot @ syn_bias).T
    onehotT = di("onehotT", [D, S], BF16)
    woT = di("woT", [D, HID], F32R)     # wo[:, hslice].T
    idm = di("idm", [128, 128], F32R)   # identity for TensorE transpose
    outT = nc.dram_tensor("outT", [HID, S], F32, kind="ExternalOutput").ap()
    lT = nc.dram_tensor("lT", [1, S], F32, kind="ExternalOutput").ap()

    with tile.TileContext(nc) as tc:
        _body(tc, xT, wq2, wk2, wv2, bias3, bsynT, onehotT, woT, idm,
              outT, lT)
    nc.compile()
    return nc


def _body(tc, xT, wq2, wk2, wv2, bias3, bsynT, onehotT, woT, idm, outT, lT):
    nc = tc.nc
    mm = nc.tensor.matmul

    with (
        tc.tile_pool(name="const", bufs=1) as constp,
        tc.tile_pool(name="big", bufs=1) as bigp,
        tc.tile_pool(name="pt", bufs=6) as ptp,
        tc.tile_pool(name="ob", bufs=2) as obp,
    ):
        # ---- constants ----
        wq_sb = constp.tile([128, 2, D], BF16, name="wq_sb", tag="wq_sb")
        wk_sb = constp.tile([128, 2, D], BF16, name="wk_sb", tag="wk_sb")
        wv_sb = constp.tile([128, 2, DV], BF16, name="wv_sb", tag="wv_sb")
        b3_sb = constp.tile([DV, 3], F32, name="b3_sb", tag="b3_sb")
        wo_sb = constp.tile([D, HID], F32R, name="wo_sb", tag="wo_sb")
        id_sb = constp.tile([128, 128], F32R, name="id_sb", tag="id_sb")

        # persistent activations
        xTa = bigp.tile([128, 2, S], BF16, name="xTa", tag="xTa")
        qTt = bigp.tile([128, S], BF16, name="qTt", tag="qTt")  # 0:64 q/8, 64:128 bsynT
        kTt = bigp.tile([128, S], BF16, name="kTt", tag="kTt")  # 0:64 k,   64:128 onehotT
        vTs = bigp.tile([DV, S], F32R, name="vTs", tag="vTs")   # v'^T (d-major)
        vb = bigp.tile([128, NKT, VBW], FP8, name="vb", tag="vb")  # v' key-major
        oall = bigp.tile([D + 1, S], F32R, name="oall", tag="oall")  # [O^T | l]

        # Few fat DMAs (Sync issues triggers at ~650ns each, so batch).
        # Order = need order: first x chunk + k weights, rest of x, the rest.
        nc.sync.dma_start(xTa[:, :, 0:2 * QB], xT[:, :, 0:2 * QB])
        nc.sync.dma_start(wk_sb[:], wk2[:])
        nc.sync.dma_start(b3_sb[:], bias3[:])
        nc.sync.dma_start(xTa[:, :, 2 * QB:5 * QB], xT[:, :, 2 * QB:5 * QB])
        nc.sync.dma_start(xTa[:, :, 5 * QB:S], xT[:, :, 5 * QB:S])
        nc.sync.dma_start(wq_sb[:], wq2[:])
        nc.sync.dma_start(wv_sb[:], wv2[:])
        nc.sync.dma_start(id_sb[:], idm[:])
        nc.sync.dma_start(kTt[64:128, :], onehotT[:])
        nc.sync.dma_start(qTt[64:128, :], bsynT[:])
        nc.sync.dma_start(wo_sb[:], woT[:])

        # zero vb's padding columns (GPSIMD: idle engine, runs under phase A)
        nc.gpsimd.memset(vb[:, :, :], 0.0)

        bq_ap = b3_sb[0:D, 0:1]
        bk_ap = b3_sb[0:D, 1:2]
        bv1_ap = b3_sb[0:DV, 2:3]

        # ---- phase A: QKV projections ----
        # K first (scores need all of kTt), each stationary loaded once per
        # two chunks, evictions split DVE (k,q) / ACT (v,vb).
        with tc.tile_pool(name="psProj", bufs=6, space="PSUM") as psP, \
             tc.tile_pool(name="psTr", bufs=2, space="PSUM") as psT:

            def proj_pair(w_sb, p, width):
                ps = []
                for t in (2 * p, 2 * p + 1):
                    sl = slice(t * QB, (t + 1) * QB)
                    pp = psP.tile([DV, QB], F32, name="pp", tag="pp")
                    mm(pp[0:width, :], w_sb[:, 0, 0:width], xTa[:, 0, sl],
                       start=True, stop=False)
                    ps.append((pp, sl))
                for pp, sl in ps:
                    mm(pp[0:width, :], w_sb[:, 1, 0:width], xTa[:, 1, sl],
                       start=False, stop=True)
                return ps

            for p in range(NQB // 2):
                for pp, sl in proj_pair(wk_sb, p, D):
                    nc.vector.tensor_scalar_add(kTt[0:D, sl], pp[0:D, :], bk_ap)
            for p in range(NQB // 2):
                for pp, sl in proj_pair(wq_sb, p, D):
                    nc.vector.tensor_scalar_add(qTt[0:D, sl], pp[0:D, :], bq_ap)
            # all v matmuls + evictions first, then a dense transpose stream
            # (keeps the PE from stalling on per-chunk ACT evictions)
            for p in range(NQB // 2):
                for pp, sl in proj_pair(wv_sb, p, DV):
                    # bias column [bv | 1 | 0..] also creates the ones row
                    nc.scalar.add(vTs[:, sl], pp[:], bv1_ap)
            for t in range(NQB):
                # flip v' to key-major: 4 TensorE transposes into one
                # PSUM tile, single fp8-converting eviction
                vtr = psT.tile([KT, 4, DV], F32R, name="vtr", tag="vtr")
                for m in range(4):
                    j = 4 * t + m
                    jl = slice(j * KT, (j + 1) * KT)
                    nc.tensor.transpose(vtr[:, m, :], vTs[:, jl],
                                        id_sb[0:DV, 0:DV])
                nc.scalar.copy(vb[:, 4 * t:4 * t + 4, 0:DV], vtr[:, :, :])

        # ---- phase B: flash attention ----
        # Pair-groups of 2 key tiles: scores land in a 2-bank PSUM tile,
        # one exp op covers both, and the PV matmul contracts both key
        # tiles at once via fp8 DoubleRow. exp alternates ACT / DVE.
        # Software pipeline: PV of group g runs after scores of g+PIPE.
        with (
            tc.tile_pool(name="psB", bufs=3, space="PSUM") as psB,
            tc.tile_pool(name="psAcc", bufs=2, space="PSUM") as psAcc,
        ):
            oaccs = {}

            def emit_pv(qb, g, p3):
                qsl = slice(qb * QB, (qb + 1) * QB)
                if g == 0:
                    oaccs[qb] = psAcc.tile([128, QB], F32, name="oacc",
                                           tag="oacc")
                oacc = oaccs[qb]
                mm(oacc[:], vb[:, 2 * g:2 * g + 2, :], p3[:, :, :],
                   start=(g == 0), stop=(g == NPAIR - 1), perf_mode=DR)
                if g == NPAIR - 1:
                    # stash [O^T | l] (normalization happens on the host),
                    # then project this block and ship it out
                    nc.scalar.copy(oall[:, qsl], oacc[0:D + 1, :])
                    pj = psB.tile([128, 2, QB], F32, name="pj", tag="s3")
                    mm(pj[:, 0, :], wo_sb[:, 0:128], oall[0:D, qsl],
                       start=True, stop=True)
                    mm(pj[:, 1, :], wo_sb[:, 128:256], oall[0:D, qsl],
                       start=True, stop=True)
                    # evict the two projection halves on different engines so
                    # each output DMA can start as soon as its half lands
                    ob = obp.tile([128, 2, QB], F32, name="ob", tag="ob")
                    nc.scalar.copy(ob[:, 0, :], pj[:, 0, :])
                    nc.sync.dma_start(outT[0:128, qsl], ob[:, 0, :])
                    nc.vector.tensor_copy(ob[:, 1, :], pj[:, 1, :])
                    nc.sync.dma_start(outT[128:256, qsl], ob[:, 1, :])

            pending = []
            for qb in range(NQB):
                qsl = slice(qb * QB, (qb + 1) * QB)
                for g in range(NPAIR):
                    s3 = psB.tile([128, 2, QB], F32, name="s3", tag="s3")
                    for i in (0, 1):
                        j = 2 * g + i
                        mm(s3[:, i, :], kTt[:, j * KT:(j + 1) * KT],
                           qTt[:, qsl], start=True, stop=True)
                    p3 = ptp.tile([128, 2, QB], FP8, name="p3", tag="p3")
                    if (qb * NPAIR + g) % 2 == 0:
                        nc.scalar.activation(p3[:, :, :], s3[:, :, :], Exp)
                    else:
                        nc.vector.tensor_scalar(
                            p3[:, :, :].bitcast(U8), s3[:, :, :],
                            float(SCH_A), float(SCH_C), MULT, ADD)
                    pending.append((qb, g, p3))
                    if len(pending) > PIPE:
                        emit_pv(*pending.pop(0))
            while pending:
                emit_pv(*pending.pop(0))

            nc.sync.dma_start(lT[:], oall[D:D + 1, :].bitcast(F32))


_NC_CACHE = {}


def _get_program():
    if "nc" not in _NC_CACHE:
        _NC_CACHE["nc"] = build_program()
    return _NC_CACHE["nc"]


def make_in_maps(x, codons, syn_bias, wq, bq, wk, bk, wv, bv, wo):
    in_maps = []
    for core in range(8):
        b, h = divmod(core, NH)
        hsl = slice(h * D, (h + 1) * D)
        cod = codons[b]
        onehotT = np.zeros((D, S), np.float32)
        onehotT[cod, np.arange(S)] = 1.0
        bias3 = np.zeros((DV, 3), np.float32)
        bias3[0:D, 0] = bq[hsl] / 8.0
        bias3[0:D, 1] = bk[hsl]
        bias3[0:D, 2] = bv[hsl]
        bias3[D, 2] = 1.0
        in_maps.append({
            "xT": split_hid(to_bf16(x[b].T)),
            "wq2": split_hid(to_bf16(wq[hsl, :].T / 8.0)),
            "wk2": split_hid(to_bf16(wk[hsl, :].T)),
            "wv2": split_hid(to_bf16(np.concatenate(
                [wv[hsl, :].T, np.zeros((HID, 4), np.float32)], axis=1))),
            "bias3": bias3,
            "bsynT": to_bf16(syn_bias.T[:, cod]),
            "onehotT": to_bf16(onehotT),
            "woT": round_fp32r(wo[:, hsl].T),
            "idm": np.eye(128, dtype=np.float32),
        })
    return in_maps


def kernel_run(inputs, trace=False):
    x = np.asarray(inputs["x"], np.float32)
    codons = np.asarray(inputs["codons"]).astype(np.int64)
    syn_bias = np.asarray(inputs["syn_bias"], np.float32)
    wq = np.asarray(inputs["wq"], np.float32)
    bq = np.asarray(inputs["bq"], np.float32)
    wk = np.asarray(inputs["wk"], np.float32)
    bk = np.asarray(inputs["bk"], np.float32)
    wv = np.asarray(inputs["wv"], np.float32)
    bv = np.asarray(inputs["bv"], np.float32)
    wo = np.asarray(inputs["wo"], np.float32)
    bo = np.asarray(inputs["bo"], np.float32)

    nc = _get_program()
    in_maps = make_in_maps(x, codons, syn_bias, wq, bq, wk, bk, wv, bv, wo)
    res = run_bass_kernel_spmd(nc, in_maps, core_ids=list(range(8)), trace=trace)

    out = np.empty((B, S, HID), np.float32)
    for b in range(B):
        acc = None
        for h in range(NH):
            r = res.results[NH * b + h]
            part = r["outT"] / r["lT"]          # normalize per head
            acc = part if acc is None else acc + part
        out[b] = acc.T + bo
    return out, res


def kernel(**inputs):
    out, _ = kernel_run(inputs, trace=False)
    return out
